# revision 1
# baseline (speedup 1.0000x reference)
"""Trainium2 Bass kernel for sliding-window GQA attention (nn_Attention_12610023981270).

Sharding: 8 cores, head-parallel — core i owns q-heads {2i, 2i+1} and kv-head i
for projections + attention, then an AllToAll switches to sequence-parallel for
the output projection (core i produces output rows [256*i, 256*(i+1))).

Everything on-chip stays "transposed" ([feature, token]) so the only transposes
needed are x itself (PE transpose-mode), and matmuls run in float32r
(full-rate ~1.6e-4 rel-err fp32 mode of the PE).

Model: B=1, T=2048, D=3584, 16 q-heads / 8 kv-heads, head_dim 256,
RoPE, query_scale 1/16, logit softcap 50, causal + sliding window 1024.
"""
import sys

if '/opt/trn_rl_repo' not in sys.path:
    sys.path.insert(0, '/opt/trn_rl_repo')

import numpy as np

import concourse.bass as bass
import concourse.mybir as mybir
import concourse.tile as tile
from concourse import bacc
from concourse.bass_utils import run_bass_kernel_spmd

f32 = mybir.dt.float32
f32r = mybir.dt.float32r
i32 = mybir.dt.int32
AF = mybir.ActivationFunctionType
Alu = mybir.AluOpType

N_CORES = 8
T, D, HD = 2048, 3584, 256
DC = D // 128            # 28 d-chunks
TWO_PI = 6.283185307179586
HALF_PI = 1.5707963267948966
SOFT_CAP = 50.0
QUERY_SCALAR = 0.0625
WINDOW = 1024
MASK_VAL = -1.0e6
TANH_SCALE = QUERY_SCALAR / SOFT_CAP   # folds query scaling into the softcap

# per-tb512 live s-chunks and the additive-mask pattern offsets
CAUSAL_DD = (0, 128, 256, 384)
WINDOW_DD = (-1024, -896, -768, -640)


def _live_chunks(tb):
    t0 = tb * 512
    smin = max(0, t0 - (WINDOW - 1))
    smax = t0 + 511
    return list(range(smin // 128, smax // 128 + 1))


def _build_module():
    nc = bacc.Bacc("TRN2", target_bir_lowering=False, debug=False,
                   num_devices=N_CORES)

    x_in = nc.declare_dram_parameter("x", [T, D], f32, isOutput=False)
    pos_in = nc.declare_dram_parameter("pos", [1, T], i32, isOutput=False)
    wq_in = nc.declare_dram_parameter("wq", [D, 512], f32, isOutput=False)
    wk_in = nc.declare_dram_parameter("wk", [D, 256], f32, isOutput=False)
    wv_in = nc.declare_dram_parameter("wv", [D, 256], f32, isOutput=False)
    wo_in = nc.declare_dram_parameter("wo", [4096, D], f32, isOutput=False)
    # consts: [:, 0:128] identity, [:, 128] ones, [:, 129] inv_timescale
    consts_in = nc.declare_dram_parameter("consts", [128, 130], f32, isOutput=False)
    out_ext = nc.declare_dram_parameter("out", [T // N_CORES, D], f32, isOutput=True)

    qT_d = nc.dram_tensor("qT_d", [512, T], f32)
    kT_d = nc.dram_tensor("kT_d", [256, T], f32)
    v_d = nc.dram_tensor("v_d", [T, 256], f32)
    cc_in = [nc.dram_tensor(f"cc_in{h}", [8, 256, 256], f32) for h in range(2)]
    cc_out = [nc.dram_tensor(f"cc_out{h}", [8, 256, 256], f32) for h in range(2)]

    with tile.TileContext(nc) as tc:
        with tc.tile_pool(name="prep", bufs=1) as prep:
            ident_r = prep.tile([128, 128], f32r)
            nc.sync.dma_start(ident_r[:], consts_in[:, 0:128].bitcast(f32r))
            ones_col_r = prep.tile([128, 1], f32r)
            nc.sync.dma_start(ones_col_r[:], consts_in[:, 128:129].bitcast(f32r))
            ones_row_f = prep.tile([1, 128], f32)
            nc.sync.dma_start(ones_row_f[:],
                              consts_in[:, 128:129].rearrange("p one -> one p"))
            ones_row_r = prep.tile([1, 128], f32r)
            nc.sync.dma_start(
                ones_row_r[:],
                consts_in[:, 128:129].rearrange("p one -> one p").bitcast(f32r))
            inv_ts = prep.tile([128, 1], f32)
            nc.sync.dma_start(inv_ts[:], consts_in[:, 129:130])

            # ---------- phase 0: RoPE sin/cos tables [128, T] ----------
            with tc.tile_pool(name="tables", bufs=1) as tbl:
                sin_t = tbl.tile([128, T], f32)
                cos_t = tbl.tile([128, T], f32)
                with (
                    tc.tile_pool(name="p0", bufs=1) as p0,
                    tc.tile_pool(name="ps0", bufs=2, space="PSUM") as ps0,
                ):
                    pos_i = p0.tile([1, T], i32)
                    nc.sync.dma_start(pos_i[:], pos_in[:])
                    pos_f = p0.tile([1, T], f32)
                    nc.vector.tensor_copy(pos_f[:], pos_i[:])
                    theta = p0.tile([128, T], f32)
                    for b in range(T // 512):
                        ps = ps0.tile([128, 512], f32, tag="bc0")
                        nc.tensor.matmul(ps[:], ones_row_f[:],
                                         pos_f[:, b * 512:(b + 1) * 512],
                                         start=True, stop=True)
                        nc.vector.tensor_scalar(theta[:, b * 512:(b + 1) * 512],
                                                ps[:], inv_ts[:], None, Alu.mult)

                    def range_reduce(dst, pre_add):
                        u = p0.tile([128, T], f32, tag="rr_u")
                        nc.vector.tensor_scalar(u[:], theta[:], pre_add,
                                                1.0 / TWO_PI, Alu.add, Alu.mult)
                        k_i = p0.tile([128, T], i32, tag="rr_k")
                        nc.vector.tensor_copy(k_i[:], u[:])
                        k_f = p0.tile([128, T], f32, tag="rr_kf")
                        nc.vector.tensor_copy(k_f[:], k_i[:])
                        r = p0.tile([128, T], f32, tag="rr_r")
                        nc.vector.tensor_tensor(r[:], u[:], k_f[:], Alu.subtract)
                        nc.vector.tensor_scalar(dst[:], r[:], TWO_PI, None,
                                                Alu.mult)

                    th_r = p0.tile([128, T], f32, tag="th_r")
                    range_reduce(th_r, 0.0)
                    nc.scalar.activation(sin_t[:], th_r[:], AF.Sin)
                    th_r2 = p0.tile([128, T], f32, tag="th_r")
                    range_reduce(th_r2, HALF_PI)
                    nc.scalar.activation(cos_t[:], th_r2[:], AF.Sin)

                # ---------- phase 1: projections ----------
                with (
                    tc.tile_pool(name="w", bufs=1) as wpool,
                    tc.tile_pool(name="xt", bufs=1) as xtp,
                    tc.tile_pool(name="xin", bufs=2) as xin,
                    tc.tile_pool(name="rope", bufs=4) as rope,
                    tc.tile_pool(name="ps_tr", bufs=2, space="PSUM") as ps_tr,
                    tc.tile_pool(name="ps_mm", bufs=1, space="PSUM") as ps_mm,
                ):
                    # per-d weight tiles: [0:512 wq | 512:768 wk | 768:1024 wv]
                    w_d = []
                    for d in range(DC):
                        wt = wpool.tile([128, 1024], f32r, tag=f"w{d}",
                                        name=f"w{d}")
                        rs = slice(d * 128, (d + 1) * 128)
                        nc.sync.dma_start(wt[:, 0:512], wq_in[rs, :].bitcast(f32r))
                        nc.sync.dma_start(wt[:, 512:768], wk_in[rs, :].bitcast(f32r))
                        nc.sync.dma_start(wt[:, 768:1024], wv_in[rs, :].bitcast(f32r))
                        w_d.append(wt)

                    for tb in range(T // 256):          # 8 t-blocks of 256
                        ts0 = tb * 256
                        # transpose x[t-block] -> xT_buf [128 D, 256 T] per d-chunk
                        xT_buf = xtp.tile([128, DC * 256], f32r, tag="xT")
                        for half in range(2):           # x tile halves [128, 1792]
                            for tc2 in range(2):        # two 128-row t-chunks
                                x_t = xin.tile([128, DC // 2 * 128], f32r,
                                               tag=f"x{tc2}")
                                nc.sync.dma_start(
                                    x_t[:],
                                    x_in[ts0 + tc2 * 128:ts0 + (tc2 + 1) * 128,
                                         half * 1792:(half + 1) * 1792].bitcast(f32r))
                                for dd_ in range(DC // 2):
                                    d = half * (DC // 2) + dd_
                                    tp = ps_tr.tile([128, 128], f32r, tag="tr")
                                    nc.tensor.transpose(
                                        tp[:], x_t[:, dd_ * 128:(dd_ + 1) * 128],
                                        ident_r[:])
                                    nc.vector.tensor_copy(
                                        xT_buf[:, d * 256 + tc2 * 128:
                                               d * 256 + tc2 * 128 + 128], tp[:])

                        # v projection: out [T-part, H-free] — lhsT = xT chunk [D, T128],
                        # rhs = wv [D, 256]; two t-chunks packed 2-up in one bank
                        ps_v = ps_mm.tile([128, 512], f32, tag="psv")
                        for tc2 in range(2):
                            for d in range(DC):
                                nc.tensor.matmul(
                                    ps_v[:, tc2 * 256:(tc2 + 1) * 256],
                                    xT_buf[:, d * 256 + tc2 * 128:
                                           d * 256 + tc2 * 128 + 128],
                                    w_d[d][:, 768:1024],
                                    start=(d == 0), stop=(d == DC - 1))

                        for tc2 in range(2):
                            v_sb = rope.tile([128, 256], f32, tag="v_sb")
                            nc.vector.tensor_copy(v_sb[:],
                                                  ps_v[:, tc2 * 256:(tc2 + 1) * 256])
                            nc.sync.dma_start(
                                v_d[ts0 + tc2 * 128:ts0 + (tc2 + 1) * 128, :], v_sb[:])

                        # qT / kT projections + RoPE
                        cos_s = cos_t[:, ts0:ts0 + 256]
                        sin_s = sin_t[:, ts0:ts0 + 256]
                        for pi_, w_off in enumerate((0, 256, 512)):  # q0, q1, k
                            ps_qk = ps_mm.tile([128, 512], f32, tag=f"qk{pi_}")
                            for c in range(2):
                                for d in range(DC):
                                    nc.tensor.matmul(
                                        ps_qk[:, c * 256:(c + 1) * 256],
                                        w_d[d][:, w_off + c * 128:
                                              w_off + c * 128 + 128],
                                        xT_buf[:, d * 256:(d + 1) * 256],
                                        start=(d == 0), stop=(d == DC - 1))
                            pA = ps_qk[:, 0:256]
                            pB = ps_qk[:, 256:512]
                            t1 = rope.tile([128, 256], f32, tag="t1")
                            t2 = rope.tile([128, 256], f32, tag="t2")
                            rot0 = rope.tile([128, 256], f32, tag="rot0")
                            rot1 = rope.tile([128, 256], f32, tag="rot1")
                            nc.vector.tensor_tensor(t1[:], pA, cos_s, Alu.mult)
                            nc.vector.tensor_tensor(t2[:], pB, sin_s, Alu.mult)
                            nc.vector.tensor_tensor(rot0[:], t1[:], t2[:], Alu.subtract)
                            nc.vector.tensor_tensor(t1[:], pB, cos_s, Alu.mult)
                            nc.vector.tensor_tensor(t2[:], pA, sin_s, Alu.mult)
                            nc.vector.tensor_tensor(rot1[:], t1[:], t2[:], Alu.add)
                            if pi_ < 2:  # q heads
                                base = pi_ * 256
                                nc.sync.dma_start(qT_d[base:base + 128, ts0:ts0 + 256],
                                                  rot0[:])
                                nc.sync.dma_start(qT_d[base + 128:base + 256,
                                                       ts0:ts0 + 256], rot1[:])
                            else:        # k
                                nc.sync.dma_start(kT_d[0:128, ts0:ts0 + 256], rot0[:])
                                nc.sync.dma_start(kT_d[128:256, ts0:ts0 + 256], rot1[:])

            # ---------- phase 2: attention ----------
            with (
                tc.tile_pool(name="kv", bufs=1) as kv,
                tc.tile_pool(name="msk", bufs=1) as msk,
                tc.tile_pool(name="qp", bufs=2) as qp,
                tc.tile_pool(name="pp", bufs=4) as pp,
                tc.tile_pool(name="enc", bufs=2) as encp,
                tc.tile_pool(name="ps_l", bufs=3, space="PSUM") as ps_lp,
                tc.tile_pool(name="ps_e", bufs=1, space="PSUM") as ps_ep,
                tc.tile_pool(name="ps_b", bufs=1, space="PSUM") as ps_bp,
            ):
                # masks (additive, applied pre-softcap)
                mask_tiles = {}
                for dd in CAUSAL_DD:
                    m = msk.tile([128, 512], f32, tag=f"mc{dd}")
                    nc.gpsimd.memset(m[:], 0.0)
                    # live iff i - j + dd <= 0  ⟺  j - i - dd >= 0
                    nc.gpsimd.affine_select(
                        out=m[:], in_=m[:], compare_op=Alu.is_ge, fill=MASK_VAL,
                        base=-dd, pattern=[[1, 512]], channel_multiplier=-1)
                    mask_tiles[dd] = m
                for dd in WINDOW_DD:
                    m = msk.tile([128, 512], f32, tag=f"mw{dd}")
                    nc.gpsimd.memset(m[:], 0.0)
                    nc.gpsimd.affine_select(
                        out=m[:], in_=m[:], compare_op=Alu.is_gt, fill=MASK_VAL,
                        base=dd + WINDOW, pattern=[[-1, 512]], channel_multiplier=1)
                    mask_tiles[dd] = m

                kT_c = []
                for c in range(2):
                    kt = kv.tile([128, T], f32r, tag=f"kt{c}")
                    nc.sync.dma_start(kt[:], kT_d[c * 128:(c + 1) * 128, :].bitcast(f32r))
                    kT_c.append(kt)
                v_all = kv.tile([128, 16 * 256], f32r, tag="v_all")
                for sj in range(16):
                    nc.sync.dma_start(v_all[:, sj * 256:(sj + 1) * 256],
                                      v_d[sj * 128:(sj + 1) * 128, :].bitcast(f32r))

                for lh in range(2):
                    for tb in range(T // 512):
                        js = _live_chunks(tb)
                        q_c = []
                        for c in range(2):
                            qt = qp.tile([128, 512], f32r, tag=f"q{c}")
                            nc.sync.dma_start(
                                qt[:],
                                qT_d[lh * 256 + c * 128:lh * 256 + (c + 1) * 128,
                                     tb * 512:(tb + 1) * 512].bitcast(f32r))
                            q_c.append(qt)
                        e0 = ps_ep.tile([128, 512], f32, tag="e0")
                        e1 = ps_ep.tile([128, 512], f32, tag="e1")
                        den = ps_ep.tile([1, 512], f32, tag="den")
                        for idx, sj in enumerate(js):
                            ps_l = ps_lp.tile([128, 512], f32, tag="l")
                            nc.tensor.matmul(ps_l[:], kT_c[0][:, sj * 128:(sj + 1) * 128],
                                             q_c[0][:], start=True, stop=False)
                            nc.tensor.matmul(ps_l[:], kT_c[1][:, sj * 128:(sj + 1) * 128],
                                             q_c[1][:], start=False, stop=True)
                            dd = sj * 128 - tb * 512
                            if dd in mask_tiles:
                                nc.vector.tensor_tensor(ps_l[:], ps_l[:],
                                                        mask_tiles[dd][:], Alu.add)
                            tmp = pp.tile([128, 512], f32, tag="tmp")
                            nc.scalar.activation(tmp[:], ps_l[:], AF.Tanh,
                                                 scale=TANH_SCALE)
                            pj = pp.tile([128, 512], f32r, tag="pj")
                            nc.scalar.activation(pj[:], tmp[:], AF.Exp,
                                                 scale=SOFT_CAP)
                            first, last = idx == 0, idx == len(js) - 1
                            nc.tensor.matmul(e0[:], v_all[:, sj * 256:sj * 256 + 128],
                                             pj[:], start=first, stop=last)
                            nc.tensor.matmul(e1[:], v_all[:, sj * 256 + 128:sj * 256 + 256],
                                             pj[:], start=first, stop=last)
                            nc.tensor.matmul(den[:], ones_col_r[:], pj[:],
                                             start=first, stop=last)
                        recip = encp.tile([1, 512], f32r, tag="recip")
                        with nc.allow_low_precision(reason="f32r recip bcast"):
                            nc.vector.reciprocal(recip[:], den[:])
                        bc = ps_bp.tile([128, 512], f32, tag="bc")
                        nc.tensor.matmul(bc[:], ones_row_r[:], recip[:],
                                         start=True, stop=True)
                        bc_sb = encp.tile([128, 512], f32, tag="bc_sb")
                        nc.scalar.copy(bc_sb[:], bc[:])
                        for c, e_ps in enumerate((e0, e1)):
                            e_sb = encp.tile([128, 512], f32, tag=f"e_sb{c}")
                            nc.vector.tensor_tensor(e_sb[:], e_ps[:], bc_sb[:],
                                                    Alu.mult)
                            r0 = c * 128
                            nc.sync.dma_start(cc_in[lh][tb * 2, r0:r0 + 128, :],
                                              e_sb[:, 0:256])
                            nc.sync.dma_start(cc_in[lh][tb * 2 + 1, r0:r0 + 128, :],
                                              e_sb[:, 256:512])

                    nc.gpsimd.collective_compute(
                        "AllToAll", Alu.bypass,
                        replica_groups=[list(range(N_CORES))],
                        ins=[cc_in[lh][:]], outs=[cc_out[lh][:]])

            # ---------- phase 3: output projection for the local T-slice ----------
            with (
                tc.tile_pool(name="ge", bufs=1) as ge,
                tc.tile_pool(name="wo", bufs=2) as wop,
                tc.tile_pool(name="o", bufs=2) as op_,
                tc.tile_pool(name="ps_o", bufs=4, space="PSUM") as ps_op,
            ):
                # per-chunk enc tiles, lh-first order so phase 3 starts
                # right after the first AllToAll; global row = src*512+lh*256+ph*128
                ORDER = [(lh, src, ph) for lh in range(2) for src in range(8)
                         for ph in range(2)]
                enc_c = []
                for i, (lh, src, ph) in enumerate(ORDER):
                    ec = ge.tile([128, 256], f32r, tag=f"e{i}", name=f"e{i}")
                    nc.sync.dma_start(
                        ec[:],
                        cc_out[lh][src, ph * 128:(ph + 1) * 128, :].bitcast(f32r))
                    enc_c.append(ec)
                for db in range(D // 512):
                    wo_buf = wop.tile([128, 32 * 512], f32r, tag="wo")
                    for i, (lh, src, ph) in enumerate(ORDER):
                        grow = (src * 4 + lh * 2 + ph) * 128
                        nc.sync.dma_start(
                            wo_buf[:, i * 512:(i + 1) * 512],
                            wo_in[grow:grow + 128,
                                  db * 512:(db + 1) * 512].bitcast(f32r))
                    for tc2 in range(2):
                        ps_o = ps_op.tile([128, 512], f32, tag="o")
                        for i in range(32):
                            nc.tensor.matmul(
                                ps_o[:],
                                enc_c[i][:, tc2 * 128:(tc2 + 1) * 128],
                                wo_buf[:, i * 512:(i + 1) * 512],
                                start=(i == 0), stop=(i == 31))
                        o_sb = op_.tile([128, 512], f32, tag="o_sb")
                        nc.vector.tensor_copy(o_sb[:], ps_o[:])
                        nc.sync.dma_start(
                            out_ext[tc2 * 128:(tc2 + 1) * 128,
                                    db * 512:(db + 1) * 512], o_sb[:])

    nc.compile()
    return nc


_CACHE = {}
LAST_RESULTS = None


def _get_module():
    if "nc" not in _CACHE:
        _CACHE["nc"] = _build_module()
    return _CACHE["nc"]


def kernel(x, segment_pos, attn_mask, wq, wkv, wo):
    global LAST_RESULTS
    x = np.asarray(x, dtype=np.float32)
    segment_pos = np.asarray(segment_pos, dtype=np.int32)
    wq = np.asarray(wq, dtype=np.float32)
    wkv = np.asarray(wkv, dtype=np.float32)
    wo = np.asarray(wo, dtype=np.float32)

    nc = _get_module()

    consts = np.zeros((128, 130), dtype=np.float32)
    consts[:, 0:128] = np.eye(128, dtype=np.float32)
    consts[:, 128] = 1.0
    consts[:, 129] = (10000.0 ** (-np.arange(128) / 128.0)).astype(np.float32)

    x2d = np.ascontiguousarray(x[0])
    pos = np.ascontiguousarray(segment_pos[0:1])
    wo_flat = np.ascontiguousarray(wo.reshape(4096, D))

    in_maps = []
    for i in range(N_CORES):
        in_maps.append({
            "x": x2d,
            "pos": pos,
            "wq": np.ascontiguousarray(
                np.concatenate([wq[2 * i], wq[2 * i + 1]], axis=1)),
            "wk": np.ascontiguousarray(wkv[0, i]),
            "wv": np.ascontiguousarray(wkv[1, i]),
            "wo": wo_flat,
            "consts": consts,
        })

    LAST_RESULTS = run_bass_kernel_spmd(nc, in_maps,
                                        core_ids=list(range(N_CORES)))
    out = np.concatenate([LAST_RESULTS.results[i]["out"]
                          for i in range(N_CORES)], axis=0)
    return out[None, :, :].astype(np.float32)



# revision 2
# speedup vs baseline: 1.0012x; 1.0012x over previous
"""Trainium2 Bass kernel v3 for sliding-window GQA attention.

v6 = v5 + phase-3 wo stream alternates gpsimd/scalar DMA queues (2x stream
bandwidth; phase 3 was DMA-paced at 306ns/MM vs the 266 floor).

v5 = v4 + masks built at t=0 from the persistent pool (attention tb0 no
longer waits for mask generation behind the wo prefetch on the gpsimd queue).

v4 = v3 (541us) + reciprocal_approx_fast (the exact reciprocal cost 3.4us
per tb on the DVE FIFO, stalling the next pair's softcap) + deeper wo
prefetch (bufs 12).

v3 changes vs v2 (622us):
- custom fused DVE op: tn = (sq(l)*c - 1)*l + mask  (one V instr replaces
  Square[S] + scalar_tensor_tensor[V] + mask add[G])
- den broadcast trick: all-ones [128,128] stationary -> den replicated on all
  128 partitions in one MM; reciprocal runs [128,512] (was [1,512], 3.3us)
- phase 0 entirely off the PE (host-broadcast positions, V-only chain)
- w8[0]/w8[1] DMA issued before the xT bulk so the PE starts at ~5us
- software-pipelined attention emission: QK(p+1) emitted before PV(p) so the
  PE stream has no per-pair stall
- phase-3 wo stream issued from the gpsimd (SWDGE) queue -> no head-of-line
  blocking behind collective-gated enc loads; enc loads follow each AllToAll
"""
import sys

if '/opt/trn_rl_repo' not in sys.path:
    sys.path.insert(0, '/opt/trn_rl_repo')

import numpy as np

import concourse.bass as bass
import concourse.mybir as mybir
import concourse.tile as tile
from concourse import bacc
from concourse.bass_utils import run_bass_kernel_spmd

f32 = mybir.dt.float32
f16 = mybir.dt.float16
i32 = mybir.dt.int32
AF = mybir.ActivationFunctionType
Alu = mybir.AluOpType

N_CORES = 8
T, D, HD = 2048, 3584, 256
DC = D // 128
TWO_PI = 6.283185307179586
SOFT_CAP = 50.0
QUERY_SCALAR = 0.0625
WINDOW = 1024
MASK16 = 60000.0
SQ_C = QUERY_SCALAR * QUERY_SCALAR / 7500.0   # (l*qs)^2/7500 = l^2 * SQ_C
EXP_BIAS = -4.0

HC_PAIRS = ((0, 1, 'k'), (2, 3, 'v'), (4, 5, 'q0'), (6, 7, 'q1'))

# ---- custom DVE op: tn = (sq(in0)*s0 - 1)*in0 + in1 ------------------------
from concourse.dve_spec import Spec, Src0, Src1, C0, One, sq, lower as dve_lower
import concourse.dve_ops as dvo


def _register_softcap_op():
    name = "SOFTCAP_MASK_ANT"
    for op in dvo.OPS:
        if op.name == name:
            return op
    spec = Spec(
        body=(sq(Src0) * C0 - One) * Src0 + Src1,
        reference=lambda in0, in1, s0, s1, imm2:
            (in0.astype(np.float32) ** 2 * s0 - 1.0) * in0 + in1,
    )
    opcode = dvo._CUSTOM_DVE_ROW_BASE + len(dvo.OPS)
    assert opcode < 0x20
    shas = {}
    for ver in ("v3", "v4"):
        uops = dve_lower(spec, ver=ver)
        shas[ver] = dvo.DveOpSpec(name=name, opcode=opcode, uops=uops,
                                  rd1_en=True).sha(ver)
    op = dvo.DveOp(name, spec, subdim=False, uops_sha=shas)
    dvo.OPS.append(op)
    dvo._SUB_OPCODE_FOR_NAME[name] = opcode
    return op


SOFTCAP_OP = _register_softcap_op()


def _live_chunks(tb):
    t0 = tb * 512
    smin = max(0, t0 - (WINDOW - 1))
    smax = t0 + 511
    return list(range(smin // 128, smax // 128 + 1))


def _build_module():
    nc = bacc.Bacc("TRN2", target_bir_lowering=False, debug=False,
                   num_devices=N_CORES)

    xT_in = nc.declare_dram_parameter("xT", [D, T], f16, isOutput=False)
    w8_in = nc.declare_dram_parameter("w8", [8, 128, D], f16, isOutput=False)
    wo_in = nc.declare_dram_parameter("wo", [4096, D], f16, isOutput=False)
    posb_in = nc.declare_dram_parameter("posb", [128, T], f32, isOutput=False)
    # cf: [:,0] inv_timescale/(2*pi)
    cf_in = nc.declare_dram_parameter("cf", [128, 1], f32, isOutput=False)
    c16_in = nc.declare_dram_parameter("c16", [128, 128], f16, isOutput=False)
    out_ext = nc.declare_dram_parameter("out", [T // N_CORES, D], f16,
                                        isOutput=True)

    cc_in = [nc.dram_tensor(f"cc_in{h}", [8, 256, 256], f16) for h in range(2)]
    cc_out = [nc.dram_tensor(f"cc_out{h}", [8, 256, 256], f16)
              for h in range(2)]

    with tile.TileContext(nc) as tc:
        with tc.tile_pool(name="hold", bufs=1) as hold:
            ident16 = hold.tile([128, 128], f16)
            nc.sync.dma_start(ident16[:], c16_in[:, :])
            invts2 = hold.tile([128, 1], f32)
            nc.sync.dma_start(invts2[:], cf_in[:, 0:1])
            bias4 = hold.tile([128, 1], f32)
            nc.gpsimd.memset(bias4[:], EXP_BIAS)
            ones128 = hold.tile([128, 128], f16)
            nc.gpsimd.memset(ones128[:], 1.0)

            pair_masks = {}
            for key, (dd0, dd1, cop) in {
                0: (0, 128, 'c'), 256: (256, 384, 'c'),
                -1024: (-1024, -896, 'w'), -768: (-768, -640, 'w'),
            }.items():
                m = hold.tile([128, 1024], f16, name=f"m{key}")
                nc.gpsimd.memset(m[:], 0.0)
                for ci, dd in enumerate((dd0, dd1)):
                    sl = m[:, ci * 512:(ci + 1) * 512]
                    if cop == 'c':
                        nc.gpsimd.affine_select(
                            out=sl, in_=sl, compare_op=Alu.is_ge,
                            fill=MASK16, base=-dd,
                            pattern=[[1, 512]], channel_multiplier=-1)
                    else:
                        nc.gpsimd.affine_select(
                            out=sl, in_=sl, compare_op=Alu.is_gt,
                            fill=MASK16, base=dd + WINDOW,
                            pattern=[[-1, 512]], channel_multiplier=1)
                pair_masks[key] = m
            zmask = hold.tile([128, 1024], f16, name="mz")
            nc.gpsimd.memset(zmask[:], 0.0)

            sin_t = hold.tile([128, T], f16)
            cos_t = hold.tile([128, T], f16)
            qT_t = [hold.tile([128, T], f16, name=f"qT{c}") for c in range(4)]
            kT_t = [hold.tile([128, T], f16, name=f"kT{c}") for c in range(2)]
            v_t = [hold.tile([128, 256], f16, name=f"v{tc_}")
                   for tc_ in range(16)]

            # ---------- phase 1 + phase 0 (phase 0 is V/S-only) ----------
            with (
                tc.tile_pool(name="xt", bufs=1) as xtp,
                tc.tile_pool(name="w", bufs=2) as wpool,
                tc.tile_pool(name="pp1", bufs=2) as pp1,
                tc.tile_pool(name="p0", bufs=1) as p0,
            ):
                # w for the first pair goes out FIRST so the PE starts early
                w_t_list = {}
                for hc in (0, 1):
                    w_t = wpool.tile([128, D], f16, tag="w", name=f"w{hc}")
                    nc.sync.dma_start(w_t[:], w8_in[hc, :, :])
                    w_t_list[hc] = w_t
                xT_t = []
                for d in range(DC):
                    xt = xtp.tile([128, T], f16, name=f"xT{d}")
                    nc.sync.dma_start(xt[:], xT_in[d * 128:(d + 1) * 128, :])
                    xT_t.append(xt)

                # phase 0: sin/cos tables, no PE involvement
                for hf in range(8):
                    hs = slice(hf * 256, (hf + 1) * 256)
                    posb = p0.tile([128, 256], f32, tag="p0p")
                    nc.sync.dma_start(posb[:], posb_in[:, hs])
                    for dst, shift in ((sin_t, 0.0), (cos_t, 0.25)):
                        a = p0.tile([128, 256], f32, tag="p0a")
                        nc.vector.tensor_scalar(a[:], posb[:], invts2[:],
                                                shift, Alu.mult, Alu.add)
                        b = p0.tile([128, 256], i32, tag="p0b")
                        nc.vector.tensor_copy(b[:], a[:])
                        c = p0.tile([128, 256], f32, tag="p0c")
                        nc.vector.tensor_copy(c[:], b[:])
                        r = p0.tile([128, 256], f32, tag="p0r")
                        nc.vector.tensor_tensor(r[:], a[:], c[:], Alu.subtract)
                        nc.scalar.activation(dst[:, hs], r[:], AF.Sin,
                                             scale=TWO_PI)

                vT_sb = [pp1.tile([128, T], f16, tag=f"vT{c}", name=f"vT{c}")
                         for c in range(2)]

                with tc.tile_pool(name="ps_p", bufs=2, space="PSUM") as ps_p:
                    for hcA, hcB, kind in HC_PAIRS:
                        ps_pair = {}
                        for hc in (hcA, hcB):
                            if hc in w_t_list:
                                w_t = w_t_list[hc]
                            else:
                                w_t = wpool.tile([128, D], f16, tag="w",
                                                 name=f"w{hc}")
                                nc.sync.dma_start(w_t[:], w8_in[hc, :, :])
                            pss = [ps_p.tile([128, 512], f32, tag=f"ps{t}",
                                             name=f"ps{t}")
                                   for t in range(4)]
                            ps_pair[hc] = pss
                            for d in range(DC):
                                for t in range(4):
                                    nc.tensor.matmul(
                                        pss[t][:],
                                        w_t[:, d * 128:(d + 1) * 128],
                                        xT_t[d][:, t * 512:(t + 1) * 512],
                                        start=(d == 0), stop=(d == DC - 1))
                        if kind == 'v':
                            for t in range(4):
                                ts_ = slice(t * 512, (t + 1) * 512)
                                nc.scalar.copy(vT_sb[0][:, ts_],
                                               ps_pair[hcA][t][:])
                                nc.scalar.copy(vT_sb[1][:, ts_],
                                               ps_pair[hcB][t][:])
                        else:
                            if kind == 'k':
                                dstA, dstB = kT_t[0], kT_t[1]
                            elif kind == 'q0':
                                dstA, dstB = qT_t[0], qT_t[1]
                            else:
                                dstA, dstB = qT_t[2], qT_t[3]
                            for t in range(4):
                                ts_ = slice(t * 512, (t + 1) * 512)
                                sA = pp1.tile([128, 512], f16, tag="sA")
                                sB = pp1.tile([128, 512], f16, tag="sB")
                                nc.scalar.copy(sA[:], ps_pair[hcA][t][:])
                                nc.scalar.copy(sB[:], ps_pair[hcB][t][:])
                                cs = cos_t[:, ts_]
                                sn = sin_t[:, ts_]
                                t1 = pp1.tile([128, 512], f16, tag="t1")
                                t2 = pp1.tile([128, 512], f16, tag="t2")
                                nc.vector.tensor_tensor(t1[:], sA[:], cs,
                                                        Alu.mult)
                                nc.vector.tensor_tensor(t2[:], sB[:], sn,
                                                        Alu.mult)
                                nc.vector.tensor_tensor(dstA[:, ts_], t1[:],
                                                        t2[:], Alu.subtract)
                                nc.vector.tensor_tensor(t1[:], sB[:], cs,
                                                        Alu.mult)
                                nc.vector.tensor_tensor(t2[:], sA[:], sn,
                                                        Alu.mult)
                                nc.vector.tensor_tensor(dstB[:, ts_], t1[:],
                                                        t2[:], Alu.add)

                with tc.tile_pool(name="ps_tr", bufs=4, space="PSUM") as ps_tr:
                    for tc_ in range(16):
                        for h2 in range(2):
                            tp = ps_tr.tile([128, 128], f16, tag="tr")
                            nc.tensor.transpose(
                                tp[:],
                                vT_sb[h2][:, tc_ * 128:(tc_ + 1) * 128],
                                ident16[:])
                            nc.vector.tensor_copy(
                                v_t[tc_][:, h2 * 128:(h2 + 1) * 128], tp[:])

            # ---------- phases 2+3 ----------
            ORDER = [(lh, src, ph) for lh in range(2) for src in range(8)
                     for ph in range(2)]
            ROUNDS = ((0, (0, 1, 2, 3)), (2048, (4, 5, 6)))
            with (
                tc.tile_pool(name="wop", bufs=12) as wop,
                tc.tile_pool(name="ge", bufs=1) as ge,
            ):
                # wo tiles for both rounds; first 8 DMAs fire during attention
                # (gpsimd SWDGE queue -> no HOL blocking of/by sync-queue DMAs)
                wo_tiles = []
                wo_meta = []
                for c0, dbs in ROUNDS:
                    for i, (lh, src, ph) in enumerate(ORDER):
                        grow = (2 * src + lh) * 256 + ph * 128
                        wo_meta.append((grow, c0, len(dbs)))
                enc_tiles = {}
                n_pref = 8
                for j in range(n_pref):
                    grow, c0, ndb = wo_meta[j]
                    wo_r = wop.tile([128, 2048], f16, tag="wo", name="wo")
                    nc.gpsimd.dma_start(wo_r[:, 0:ndb * 512],
                                        wo_in[grow:grow + 128,
                                              c0:c0 + ndb * 512])
                    wo_tiles.append(wo_r)

                with (
                    tc.tile_pool(name="att", bufs=3) as att,
                    tc.tile_pool(name="attb", bufs=2) as attb,
                    tc.tile_pool(name="ps_l", bufs=2, space="PSUM") as ps_lp,
                    tc.tile_pool(name="ps_e", bufs=1, space="PSUM") as ps_ep,
                    tc.tile_pool(name="ps_d", bufs=2, space="PSUM") as ps_dp,
                ):
                    for lh in range(2):
                        qA, qB = qT_t[2 * lh], qT_t[2 * lh + 1]
                        pend = None

                        def emit_tail(p):
                            (tb, sj0, sj1, ps_pair, e0, e1, dbc,
                             first, last) = p
                            dd0 = sj0 * 128 - tb * 512
                            mk = pair_masks.get(dd0, zmask)
                            tn = att.tile([128, 1024], f16, tag="tn",
                                          name="tn")
                            nc.vector._custom_dve(SOFTCAP_OP, out=tn[:],
                                                  in0=ps_pair[:], in1=mk[:],
                                                  s0=SQ_C)
                            pj = att.tile([128, 1024], f16, tag="pj",
                                          name="pj")
                            nc.scalar.activation(pj[:], tn[:], AF.Exp,
                                                 scale=-QUERY_SCALAR,
                                                 bias=bias4[:])
                            for ci, sj in enumerate((sj0, sj1)):
                                pjc = pj[:, ci * 512:(ci + 1) * 512]
                                f = first and ci == 0
                                l = last and ci == 1
                                nc.tensor.matmul(e0[:], v_t[sj][:, 0:128],
                                                 pjc, start=f, stop=l)
                                nc.tensor.matmul(e1[:], v_t[sj][:, 128:256],
                                                 pjc, start=f, stop=l)
                                nc.tensor.matmul(dbc[:], ones128[:], pjc,
                                                 start=f, stop=l)
                            if last:
                                rec = attb.tile([128, 512], f32, tag="rec",
                                                name="rec")
                                nc.vector.reciprocal_approx_fast(rec[:],
                                                                 dbc[:])
                                for c, e_ps in enumerate((e0, e1)):
                                    e_sb = attb.tile([128, 512], f16,
                                                     tag=f"e_sb{c}",
                                                     name=f"e_sb{c}")
                                    nc.vector.tensor_tensor(
                                        e_sb[:], e_ps[:], rec[:], Alu.mult)
                                    r0 = c * 128
                                    nc.sync.dma_start(
                                        cc_in[lh][tb * 2, r0:r0 + 128, :],
                                        e_sb[:, 0:256])
                                    nc.sync.dma_start(
                                        cc_in[lh][tb * 2 + 1, r0:r0 + 128, :],
                                        e_sb[:, 256:512])

                        for tb in range(4):
                            js = _live_chunks(tb)
                            npair = len(js) // 2
                            e0 = ps_ep.tile([128, 512], f32, tag="e0",
                                            name="e0")
                            e1 = ps_ep.tile([128, 512], f32, tag="e1",
                                            name="e1")
                            dbc = ps_dp.tile([128, 512], f32, tag="dbc",
                                             name="dbc")
                            qs = slice(tb * 512, (tb + 1) * 512)
                            for pi in range(npair):
                                sj0, sj1 = js[2 * pi], js[2 * pi + 1]
                                ps_pair = ps_lp.tile([128, 1024], f32,
                                                     tag="l", name="l")
                                for ci, sj in enumerate((sj0, sj1)):
                                    ks = slice(sj * 128, (sj + 1) * 128)
                                    ls = slice(ci * 512, (ci + 1) * 512)
                                    nc.tensor.matmul(ps_pair[:, ls],
                                                     kT_t[0][:, ks],
                                                     qA[:, qs],
                                                     start=True, stop=False)
                                    nc.tensor.matmul(ps_pair[:, ls],
                                                     kT_t[1][:, ks],
                                                     qB[:, qs],
                                                     start=False, stop=True)
                                if pend is not None:
                                    emit_tail(pend)
                                pend = (tb, sj0, sj1, ps_pair, e0, e1, dbc,
                                        pi == 0, pi == npair - 1)
                        emit_tail(pend)
                        pend = None

                        nc.gpsimd.collective_compute(
                            "AllToAll", Alu.bypass,
                            replica_groups=[list(range(N_CORES))],
                            ins=[cc_in[lh][:]], outs=[cc_out[lh][:]])
                        for src in range(8):
                            for ph in range(2):
                                i = lh * 16 + src * 2 + ph
                                ec = ge.tile([128, 256], f16, tag=f"e{i}",
                                             name=f"e{i}")
                                nc.sync.dma_start(
                                    ec[:],
                                    cc_out[lh][src, ph * 128:(ph + 1) * 128,
                                               :])
                                enc_tiles[i] = ec

                # ---------- phase 3 ----------
                with tc.tile_pool(name="o", bufs=4) as op_:
                    j = n_pref
                    for ri, (c0, dbs) in enumerate(ROUNDS):
                        with tc.tile_pool(name="ps_o", bufs=1,
                                          space="PSUM") as ps_op:
                            pso = {}
                            for tc2 in range(2):
                                for db in dbs:
                                    pso[(tc2, db)] = ps_op.tile(
                                        [128, 512], f32, tag=f"o{tc2}_{db}",
                                        name=f"o{tc2}_{db}")
                            for i, (lh, src, ph) in enumerate(ORDER):
                                gi = ri * 32 + i
                                if gi < n_pref:
                                    wo_r = wo_tiles[gi]
                                else:
                                    grow, cc0, ndb = wo_meta[gi]
                                    wo_r = wop.tile([128, 2048], f16,
                                                    tag="wo", name="wo")
                                    eng = nc.gpsimd if gi % 2 == 0 else \
                                        nc.scalar
                                    eng.dma_start(
                                        wo_r[:, 0:ndb * 512],
                                        wo_in[grow:grow + 128,
                                              cc0:cc0 + ndb * 512])
                                ec = enc_tiles[lh * 16 + src * 2 + ph]
                                for tc2 in range(2):
                                    st = ec[:, tc2 * 128:(tc2 + 1) * 128]
                                    for k, db in enumerate(dbs):
                                        nc.tensor.matmul(
                                            pso[(tc2, db)][:], st,
                                            wo_r[:, k * 512:(k + 1) * 512],
                                            start=(i == 0), stop=(i == 31))
                            for tc2 in range(2):
                                for db in dbs:
                                    o_sb = op_.tile([128, 512], f16,
                                                    tag="o_sb")
                                    if tc2 == 0:
                                        nc.vector.tensor_copy(
                                            o_sb[:], pso[(tc2, db)][:])
                                    else:
                                        nc.scalar.copy(o_sb[:],
                                                       pso[(tc2, db)][:])
                                    nc.sync.dma_start(
                                        out_ext[tc2 * 128:(tc2 + 1) * 128,
                                                db * 512:(db + 1) * 512],
                                        o_sb[:])

    nc.compile()
    return nc


_CACHE = {}
LAST_RESULTS = None


def _get_module():
    if "nc" not in _CACHE:
        _CACHE["nc"] = _build_module()
    return _CACHE["nc"]


def kernel(x, segment_pos, attn_mask, wq, wkv, wo):
    global LAST_RESULTS
    x = np.asarray(x, dtype=np.float32)
    segment_pos = np.asarray(segment_pos, dtype=np.int32)
    wq = np.asarray(wq, dtype=np.float32)
    wkv = np.asarray(wkv, dtype=np.float32)
    wo = np.asarray(wo, dtype=np.float32)

    nc = _get_module()

    xT16 = np.ascontiguousarray(x[0].T.astype(np.float16))
    posb = np.ascontiguousarray(
        np.broadcast_to(segment_pos[0].astype(np.float32)[None, :],
                        (128, T)))
    wo16 = np.ascontiguousarray(wo.reshape(4096, D).astype(np.float16))

    cf = ((10000.0 ** (-np.arange(128) / 128.0)) / TWO_PI).astype(
        np.float32).reshape(128, 1)
    c16 = np.eye(128, dtype=np.float16)

    def retile(w_col):
        return np.ascontiguousarray(
            w_col.reshape(DC, 128, 128).transpose(1, 0, 2).reshape(128, D)
            .astype(np.float16))

    in_maps = []
    for i in range(N_CORES):
        wk = wkv[0, i]
        wv = wkv[1, i]
        q0 = wq[2 * i]
        q1 = wq[2 * i + 1]
        w8 = np.stack([
            retile(wk[:, 0:128]), retile(wk[:, 128:256]),
            retile(wv[:, 0:128]), retile(wv[:, 128:256]),
            retile(q0[:, 0:128]), retile(q0[:, 128:256]),
            retile(q1[:, 0:128]), retile(q1[:, 128:256]),
        ])
        in_maps.append({
            "xT": xT16,
            "w8": np.ascontiguousarray(w8),
            "wo": wo16,
            "posb": posb,
            "cf": cf,
            "c16": c16,
        })

    LAST_RESULTS = run_bass_kernel_spmd(nc, in_maps,
                                        core_ids=list(range(N_CORES)))
    out = np.concatenate([LAST_RESULTS.results[i]["out"]
                          for i in range(N_CORES)], axis=0)
    return out[None, :, :].astype(np.float32)
